# revision 7
# baseline (speedup 1.0000x reference)
"""BiLSTM-CRF Trainium2 kernel.

Structure:
  NEFF A (2 cores SPMD, core0=forward dir, core1=backward dir on host-time-reversed
  input): per-layer pre-matmul (pre = x @ W_ih.T + b, bias folded via augmented
  ones-row) + 2048-step LSTM recurrence (PE h-stationary matvec, ACT gates,
  PE transposes to keep h partition-major). Run twice (layer 1, layer 2).
  NEFF B (1 core): dense projection to 12 tags + Viterbi forward scan producing
  backpointers. Host does the O(T*K) backtrace.
"""
import sys
sys.path.insert(0, "/opt/trn_rl_repo")
import numpy as np

from concourse import bass, bacc, tile
from concourse import mybir
from concourse.bass_utils import run_bass_kernel_spmd

F32 = mybir.dt.float32
U32 = mybir.dt.uint32
AF = mybir.ActivationFunctionType
ALU = mybir.AluOpType

T = 2048
H = 1024
E = 400
K = 12
KIN = 2176          # padded input feature dim (2048 + 1 ones row -> 17 k-tiles)
NK = KIN // 128     # 17
START, STOP = 10, 11


def build_layer_nc():
    nc = bacc.Bacc(None, target_bir_lowering=False, num_devices=2)
    xT = nc.dram_tensor("xT", [KIN, T], F32, kind="ExternalInput")
    wiT = nc.dram_tensor("wiT", [KIN, 4 * H], F32, kind="ExternalInput")
    whT = nc.dram_tensor("whT", [H, 4 * H], F32, kind="ExternalInput")
    hsT = nc.dram_tensor("hsT", [H, T], F32, kind="ExternalOutput")

    with tile.TileContext(nc) as tc:
        with tc.tile_pool(name="dram", bufs=1, space="DRAM") as dram:
            pre = dram.tile([T, 4 * H], F32)

            # ---- Phase 1: pre = x @ W_ih.T (+bias via aug row) ----
            with tc.tile_pool(name="xsb", bufs=1) as xpool, \
                 tc.tile_pool(name="wblk", bufs=1) as wpool, \
                 tc.tile_pool(name="pps", bufs=4, space="PSUM") as ppool:
                xsb = xpool.tile([128, NK * T], F32)
                nc.sync.dma_start(
                    xsb[:].rearrange("p (k t) -> p k t", k=NK), xT.rearrange("(k p) t -> p k t", p=128)[:])
                for n in range(8):
                    wblk = wpool.tile([128, NK * 512], F32)
                    nc.sync.dma_start(
                        wblk[:],
                        wiT.rearrange("(k p) n -> p k n", p=128)[:, :, n * 512:(n + 1) * 512])
                    for ti in range(16):
                        ps = ppool.tile([128, 512], F32)
                        for k in range(NK):
                            nc.tensor.matmul(
                                ps[:],
                                xsb[:, k * T + ti * 128: k * T + (ti + 1) * 128],
                                wblk[:, k * 512:(k + 1) * 512],
                                start=(k == 0), stop=(k == NK - 1))
                        pb = wpool.tile([128, 512], F32, tag="pb")
                        nc.vector.tensor_copy(pb[:], ps[:])
                        nc.sync.dma_start(
                            pre[ti * 128:(ti + 1) * 128, n * 512:(n + 1) * 512], pb[:])

            # ---- Phase 2: recurrence ----
            with tc.tile_pool(name="wh", bufs=1) as whpool, \
                 tc.tile_pool(name="st", bufs=1) as stp, \
                 tc.tile_pool(name="tmp", bufs=2) as tmp, \
                 tc.tile_pool(name="pr", bufs=1) as prp, \
                 tc.tile_pool(name="zz", bufs=8, space="PSUM") as zp:
                whsb = whpool.tile([128, 8 * 4 * H], F32)
                nc.sync.dma_start(
                    whsb[:].rearrange("p (k n) -> p k n", k=8), whT.rearrange("(k p) n -> p k n", p=128)[:])
                hT = stp.tile([128, 8], F32)
                cst = stp.tile([1, H], F32)
                one = stp.tile([1, 1], F32)
                sig = stp.tile([1, 3 * H], F32)
                tg = stp.tile([1, H], F32)
                tc_t = stp.tile([1, H], F32)
                hv = stp.tile([1, H], F32)
                nc.vector.memset(hT[:], 0.0)
                nc.vector.memset(cst[:], 0.0)
                nc.vector.memset(one[:], 1.0)

                hsT_v = hsT.rearrange("(j p) t -> p j t", p=128)

                with tc.For_i(0, T, 1) as t:
                    presb = prp.tile([1, 4 * H], F32, tag="presb")
                    nc.sync.dma_start(presb[:], pre[bass.ds(t, 1), :])
                    z = [zp.tile([1, 512], F32, tag="z", name=f"z{_n}") for _n in range(8)]
                    for k in range(8):
                        for n in range(8):
                            nc.tensor.matmul(
                                z[n][:],
                                hT[:, k:k + 1],
                                whsb[:, k * 4 * H + n * 512: k * 4 * H + (n + 1) * 512],
                                start=(k == 0), stop=False)
                    for n in range(8):
                        nc.tensor.matmul(
                            z[n][:], one[:], presb[0:1, n * 512:(n + 1) * 512],
                            start=False, stop=True)
                    # gate order along 4H: [i f o g]
                    for b in range(6):
                        nc.scalar.activation(
                            sig[0:1, b * 512:(b + 1) * 512], z[b][:], AF.Sigmoid)
                    for b in range(2):
                        nc.scalar.activation(
                            tg[0:1, b * 512:(b + 1) * 512], z[6 + b][:], AF.Tanh)
                    t1 = tmp.tile([1, H], F32, tag="t1")
                    c2 = tmp.tile([1, H], F32, tag="c2")
                    nc.vector.tensor_tensor(t1[:], sig[0:1, 0:H], tg[0:1, :], ALU.mult)
                    nc.vector.tensor_tensor(c2[:], sig[0:1, H:2 * H], cst[:], ALU.mult)
                    nc.vector.tensor_tensor(cst[:], c2[:], t1[:], ALU.add)
                    nc.scalar.activation(tc_t[:], cst[:], AF.Tanh)
                    nc.vector.tensor_tensor(hv[:], sig[0:1, 2 * H:3 * H], tc_t[:], ALU.mult)
                    tp = zp.tile([128, 8], F32, tag="z")
                    for j in range(8):
                        nc.tensor.transpose(
                            tp[:, j:j + 1], hv[0:1, j * 128:(j + 1) * 128], one[:])
                    nc.vector.tensor_copy(hT[:], tp[:])
                    nc.sync.dma_start(hsT_v[:, :, bass.ds(t, 1)], hT[:])
    nc.finalize()
    return nc


def build_crf_nc():
    nc = bacc.Bacc(None, target_bir_lowering=False, num_devices=1)
    x3T = nc.dram_tensor("x3T", [KIN, T], F32, kind="ExternalInput")
    dwT = nc.dram_tensor("dwT", [KIN, K], F32, kind="ExternalInput")
    trT = nc.dram_tensor("trT", [K, K], F32, kind="ExternalInput")
    id12 = nc.dram_tensor("id12", [K, K], F32, kind="ExternalInput")
    fv0 = nc.dram_tensor("fv0", [K, 1], F32, kind="ExternalInput")
    bps = nc.dram_tensor("bps", [K, T], U32, kind="ExternalOutput")
    fvout = nc.dram_tensor("fvout", [K, 1], F32, kind="ExternalOutput")
    featsT_o = nc.dram_tensor("featsT", [K, T], F32, kind="ExternalOutput")

    with tile.TileContext(nc) as tc:
        with tc.tile_pool(name="sb", bufs=1) as sb, \
             tc.tile_pool(name="tmp", bufs=4) as tmp, \
             tc.tile_pool(name="ps", bufs=4, space="PSUM") as ps:
            x3sb = sb.tile([128, NK * T], F32)
            nc.sync.dma_start(x3sb[:].rearrange("p (k t) -> p k t", k=NK), x3T.rearrange("(k p) t -> p k t", p=128)[:])
            dwsb = sb.tile([128, NK * K], F32)
            nc.sync.dma_start(dwsb[:].rearrange("p (k n) -> p k n", k=NK), dwT.rearrange("(k p) n -> p k n", p=128)[:])
            featsT = sb.tile([K, T], F32)
            for n4 in range(4):
                fp = ps.tile([K, 512], F32, tag="fp")
                for k in range(NK):
                    nc.tensor.matmul(
                        fp[:], dwsb[:, k * K:(k + 1) * K],
                        x3sb[:, k * T + n4 * 512: k * T + (n4 + 1) * 512],
                        start=(k == 0), stop=(k == NK - 1))
                nc.vector.tensor_copy(featsT[:, n4 * 512:(n4 + 1) * 512], fp[:])
            nc.sync.dma_start(featsT_o[:], featsT[:])

            trsb = sb.tile([K, K], F32)
            nc.sync.dma_start(trsb[:], trT[:])
            idsb = sb.tile([K, K], F32)
            nc.sync.dma_start(idsb[:], id12[:])
            fvT = sb.tile([K, 1], F32)
            nc.sync.dma_start(fvT[:], fv0[:])
            bsb = sb.tile([K, T], U32)

            with tc.For_i(0, T, 1) as t:
                sc = tmp.tile([K, K], F32, tag="sc")
                nc.vector.tensor_scalar(sc[:], trsb[:], fvT[:, 0:1], None, ALU.add)
                scp = ps.tile([K, K], F32, tag="scp")
                nc.tensor.transpose(scp[:], sc[:], idsb[:])
                scs = tmp.tile([K, K], F32, tag="scs")
                nc.vector.tensor_copy(scs[:], scp[:])
                mx = tmp.tile([K, 8], F32, tag="mx")
                mi = tmp.tile([K, 8], U32, tag="mi")
                nc.vector.max(mx[:], scs[:])
                nc.vector.max_index(mi[:], mx[:], scs[:])
                nc.vector.tensor_tensor(
                    fvT[:], mx[:, 0:1], featsT[:, bass.ds(t, 1)], ALU.add)
                nc.vector.tensor_copy(bsb[:, bass.ds(t, 1)], mi[:, 0:1])
            nc.sync.dma_start(bps[:], bsb[:])
            nc.sync.dma_start(fvout[:], fvT[:])
    nc.finalize()
    return nc


_LAYER_NC = None
_CRF_NC = None


def _aug_x(xT_nat):
    """[D, T] -> [KIN, T] with ones row at index D."""
    d = xT_nat.shape[0]
    out = np.zeros((KIN, T), dtype=np.float32)
    out[:d] = xT_nat
    out[d] = 1.0
    return out


def _prep_w(w_ih, w_hh, b_ih, b_hh):
    """Return wiT [KIN, 4H] (bias in aug row, gate order [i f o g]) and whT."""
    w_ih = np.asarray(w_ih); w_hh = np.asarray(w_hh)
    b = np.asarray(b_ih) + np.asarray(b_hh)
    # reorder gates from [i f g o] to [i f o g]
    idx = np.concatenate([np.arange(0, H), np.arange(H, 2 * H),
                          np.arange(3 * H, 4 * H), np.arange(2 * H, 3 * H)])
    w_ih = w_ih[idx]; w_hh = w_hh[idx]; b = b[idx]
    d = w_ih.shape[1]
    wiT = np.zeros((KIN, 4 * H), dtype=np.float32)
    wiT[:d] = w_ih.T
    wiT[d] = b
    whT = np.ascontiguousarray(w_hh.T, dtype=np.float32)
    return wiT, whT


def _run_layer(xT_f, xT_b, params):
    global _LAYER_NC
    if _LAYER_NC is None:
        _LAYER_NC = build_layer_nc()
    wif, whf, bif, bhf, wib, whb, bib, bhb = params
    wiT_f, whT_f = _prep_w(wif, whf, bif, bhf)
    wiT_b, whT_b = _prep_w(wib, whb, bib, bhb)
    in_maps = [
        {"xT": _aug_x(xT_f), "wiT": wiT_f, "whT": whT_f},
        {"xT": _aug_x(xT_b), "wiT": wiT_b, "whT": whT_b},
    ]
    res = run_bass_kernel_spmd(_LAYER_NC, in_maps, core_ids=[0, 1])
    return res.results[0]["hsT"], res.results[1]["hsT"]


def kernel(sentence, lstm_params, dense_w, dense_b, transition):
    global _CRF_NC
    x = np.asarray(sentence, dtype=np.float32)[0]          # [T, E]
    xT = np.ascontiguousarray(x.T)                          # [E, T]
    xT_rev = np.ascontiguousarray(x[::-1].T)

    cur_f, cur_b = xT, xT_rev
    for l, params in enumerate(lstm_params):
        hf, hb_loc = _run_layer(cur_f, cur_b, params)       # [H, T] each
        hb = hb_loc[:, ::-1]                                # back to natural time
        x2T = np.concatenate([hf, hb], axis=0)              # [2H, T]
        cur_f = x2T
        cur_b = np.ascontiguousarray(x2T[:, ::-1])

    if _CRF_NC is None:
        _CRF_NC = build_crf_nc()
    dense_w = np.asarray(dense_w, dtype=np.float32)
    dense_b = np.asarray(dense_b, dtype=np.float32)
    transition = np.asarray(transition, dtype=np.float32)
    dwT = np.zeros((KIN, K), dtype=np.float32)
    dwT[:2 * H] = dense_w.T
    dwT[2 * H] = dense_b
    fv0 = np.full((K, 1), -10000.0, dtype=np.float32)
    fv0[START, 0] = 0.0
    in_map = {
        "x3T": _aug_x(cur_f), "dwT": dwT,
        "trT": np.ascontiguousarray(transition.T),
        "id12": np.eye(K, dtype=np.float32),
        "fv0": fv0,
    }
    res = run_bass_kernel_spmd(_CRF_NC, [in_map], core_ids=[0])
    bps = res.results[0]["bps"].astype(np.int64)            # [K, T]
    fv = res.results[0]["fvout"][:, 0]                      # [K]

    terminal = fv + transition[STOP]
    best = int(np.argmax(terminal))
    path_score = np.float32(terminal[best])
    path = np.empty(T, dtype=np.int32)
    tag = best
    for t in range(T - 1, -1, -1):
        path[t] = tag
        tag = int(bps[tag, t])
    return path_score, path
